# revision 7
# baseline (speedup 1.0000x reference)
"""Causal self-attention (B=4, T=2048, C=1024, H=16, D=64) on 8 TRN2 NeuronCores.

Sharding: 4 batches x 2 head-groups (8 heads each). Core c handles batch c//2,
heads 8*(c%2) .. 8*(c%2)+7. Host pre-transposes x and slices/transposes the
weights so the device kernel needs no on-chip transposes. All matmul operands
are bf16 (fp32 PSUM accumulation).

Fused single-pass schedule: the QKV projection (4 t-chunks x 12 matmul groups)
and the attention/proj work are interleaved in one stream. Attention for query
chunk qc starts as soon as projection chunk tci=qc is done; remaining
projection groups are rationed into the attention loop as PE filler so the
scalar engine (exp) and PE stay busy together instead of running as separate
phases.

  projection: qkT[feat, T] = Wqk_g @ x^T, V[t, vfeat] = x @ Wv_g^T (+ ones col)
  attention:  per head-pair (2p, 2p+1) and query chunk qc: for each k-block j,
              S^T blocks for BOTH heads land in one [128,2,512] PSUM tile via
              row-tiled matmuls (head 2p on PE tile (0,0) from SBUF partitions
              0:64, head 2p+1 on (64,0) — concurrent); ONE exp call covers the
              pair, so both heads' next scores are gated by the same semaphore
              and issue back-to-back (keeps the pairing aligned).
              P^T = exp(S^T/8) * causal masks, out^T[d|sum, q] = [V_h|1]^T P^T,
              normalized by approx-reciprocal + gpsimd partition-broadcast.
  proj:       y = attn^T.T @ Wp_g^T, interleaved between pairs as PE filler

Each core returns a [2048, 1024] partial; the host sums the two head-group
partials per batch.
"""

import numpy as np

T = 2048
N_CORES = 8

_CACHE = {}


def _build_module():
    from collections import deque
    from contextlib import ExitStack

    import concourse.tile as tile
    from concourse.tile_rust import add_dep_helper
    from concourse import bacc, mybir

    f32 = mybir.dt.float32
    bf16 = mybir.dt.bfloat16
    Exp = mybir.ActivationFunctionType.Exp
    Copy = mybir.ActivationFunctionType.Copy

    nc = bacc.Bacc("TRN2", target_bir_lowering=False, debug=False,
                   num_devices=N_CORES)

    xT_d = nc.dram_tensor("xT", (1024, 2048), bf16, kind="ExternalInput").ap()
    wqkT_d = nc.dram_tensor("wqkT", (1024, 1024), bf16, kind="ExternalInput").ap()
    wvT_d = nc.dram_tensor("wvT", (1024, 512), bf16, kind="ExternalInput").ap()
    wpT_d = nc.dram_tensor("wpT", (512, 1024), bf16, kind="ExternalInput").ap()
    mk_d = nc.dram_tensor("trimask", (128, 128), bf16, kind="ExternalInput").ap()
    y_d = nc.dram_tensor("y", (2048, 1024), f32, kind="ExternalOutput").ap()

    with tile.TileContext(nc) as tc, ExitStack() as ctx:
        pers = ctx.enter_context(tc.tile_pool(name="pers", bufs=1))
        sb_qT = pers.tile([128, 4, 2048], bf16, name="sb_qT")
        sb_kT = pers.tile([128, 4, 2048], bf16, name="sb_kT")
        sb_v = pers.tile([128, 16, 520], bf16, name="sb_v")
        v_view = sb_v[:].rearrange("p t (h e) -> p t h e", e=65)
        sb_attnT = pers.tile([128, 4, 2048], bf16, name="sb_attnT")
        sb_wpT = pers.tile([128, 4, 1024], bf16, name="sb_wpT")
        sb_mask2 = pers.tile([128, 2, 128], bf16, name="sb_mask2")
        sb_wvT = pers.tile([128, 8, 512], bf16, name="sb_wvT")

        wqk_pool = ctx.enter_context(tc.tile_pool(name="wqk", bufs=8))
        xt_pool = ctx.enter_context(tc.tile_pool(name="xt", bufs=2))
        ps_misc = ctx.enter_context(tc.tile_pool(name="ps_misc", bufs=2,
                                                 space="PSUM"))
        ps_s = ctx.enter_context(tc.tile_pool(name="ps_s", bufs=2, space="PSUM"))
        ps_o = ctx.enter_context(tc.tile_pool(name="ps_o", bufs=2, space="PSUM"))
        exp_pool = ctx.enter_context(tc.tile_pool(name="expp", bufs=3))
        norm_pool = ctx.enter_context(tc.tile_pool(name="normp", bufs=3))
        y_pool = ctx.enter_context(tc.tile_pool(name="yp", bufs=2))
        misc_sb = ctx.enter_context(tc.tile_pool(name="miscsb", bufs=1))

        wqkT_r = wqkT_d.rearrange("(co ci) f -> ci co f", ci=128)
        xT_r = xT_d.rearrange("(co ci) t -> ci co t", ci=128)
        wvT_r = wvT_d.rearrange("(co ci) f -> ci co f", ci=128)
        wpT_r = wpT_d.rearrange("(ko ki) n -> ki ko n", ki=128)

        # ---- DMA admission: first weights + x chunk 0, then chained ----
        fbs = [4, 5, 6, 7, 0, 1, 2, 3]  # K features first, then Q
        wtiles = {}
        wt = wqk_pool.tile([128, 8, 128], bf16, tag="wqk", name="wt4")
        nc.sync.dma_start(wt[:], wqkT_r[:, :, 4 * 128:5 * 128])
        wtiles[4] = wt
        xchunks = {}
        xchunks[0] = xt_pool.tile([128, 8, 512], bf16, tag="xt", name="xc0")
        xdmas = [nc.sync.dma_start(xchunks[0][:, co, :], xT_r[:, co, 0:512])
                 for co in range(8)]
        prev = xdmas[-1]
        for grp in [(5, 6), (7, 0), (1, 2), (3,)]:
            for fb in grp:
                wt = wqk_pool.tile([128, 8, 128], bf16, tag="wqk", name=f"wt{fb}")
                d = nc.sync.dma_start(wt[:], wqkT_r[:, :, fb * 128:(fb + 1) * 128])
                add_dep_helper(d.ins, prev.ins, sync=False, reason="stage w")
                wtiles[fb] = wt
            prev = d
        for co in range(8):
            d = nc.sync.dma_start(sb_wvT[:, co, :], wvT_r[:, co, :])
            add_dep_helper(d.ins, prev.ins, sync=False, reason="stage wv")
        wv_last = d

        # ones column of sb_v via exp(0)=1 — also warms the ACT exp table
        zeros = misc_sb.tile([128, 128], f32, name="zeros")
        nc.vector.memset(zeros[:], 0.0)
        nc.scalar.activation(
            v_view[:, :, :, 64:65],
            zeros[:].rearrange("p (a b c) -> p a b c", a=16, b=8),
            Exp,
        )

        # ---- projection group emitters ----
        round_copy = {}
        x_stage_prev = [wv_last]

        def stage_x(tci):
            # admit x chunk `tci` (chained behind earlier admission DMAs)
            xchunks[tci] = xt_pool.tile([128, 8, 512], bf16, tag="xt",
                                        name=f"xc{tci}")
            first = None
            for co in range(8):
                d = nc.sync.dma_start(
                    xchunks[tci][:, co, :],
                    xT_r[:, co, tci * 512:(tci + 1) * 512])
                if first is None:
                    add_dep_helper(d.ins, x_stage_prev[0].ins, sync=False,
                                   reason="stage x chunk")
                    first = d
            x_stage_prev[0] = d

        def qk_group(tci, fb):
            dst, pblk = (sb_kT, fb - 4) if fb >= 4 else (sb_qT, fb)
            ps = ps_misc.tile([128, 512], f32, tag="psb", name="psqk")
            for co in range(8):
                nc.tensor.matmul(
                    ps[:],
                    lhsT=wtiles[fb][:, co, :],
                    rhs=xchunks[tci][:, co, :],
                    start=(co == 0), stop=(co == 7),
                )
            cp = nc.vector.tensor_copy(
                dst[:, pblk, tci * 512:(tci + 1) * 512], ps[:])
            if fb == 4:
                round_copy[tci] = cp

        def v_group(tci, tb):
            tblk = tci * 4 + tb
            ps = ps_misc.tile([128, 512], f32, tag="psb", name="psv")
            for co in range(8):
                nc.tensor.matmul(
                    ps[:],
                    lhsT=xchunks[tci][:, co, tb * 128:(tb + 1) * 128],
                    rhs=sb_wvT[:, co, :],
                    start=(co == 0), stop=(co == 7),
                )
            nc.vector.tensor_copy(
                v_view[:, tblk, :, 0:64],
                ps[:].rearrange("p (h d) -> p h d", d=64),
            )

        def groups_for(tci):
            gs = []
            for fb in fbs:
                gs.append((tci, lambda t=tci, f=fb: qk_group(t, f)))
            for tb in range(4):
                gs.append((tci, lambda t=tci, b=tb: v_group(t, b)))
            return gs

        # ---- attention emitters ----
        def emit_proj(tblk, on_act=False):
            for n in range(2):
                ysb = y_pool.tile([128, 512], f32, tag="ysb", name="ysb")
                pj = ps_misc.tile([128, 512], f32, tag="psb", name="pj")
                for ko in range(4):
                    nc.tensor.matmul(
                        pj[:],
                        lhsT=sb_attnT[:, ko, tblk * 128:(tblk + 1) * 128],
                        rhs=sb_wpT[:, ko, n * 512:(n + 1) * 512],
                        start=(ko == 0), stop=(ko == 3),
                    )
                if on_act:
                    nc.scalar.activation(ysb[:], pj[:], Copy)
                else:
                    nc.vector.tensor_copy(ysb[:], pj[:])
                nc.sync.dma_start(
                    y_d[tblk * 128:(tblk + 1) * 128, n * 512:(n + 1) * 512],
                    ysb[:])

        def norm_store(po, rr, p_, qc, on_act=False):
            att_slice = sb_attnT[rr:rr + 64, p_, qc * 512:(qc + 1) * 512]
            sums = norm_pool.tile([1, 512], f32, tag="sums", name="sums")
            if on_act:
                nc.scalar.activation(att_slice, po[0:64, :], Copy)
                nc.scalar.activation(sums[:], po[64:65, :], Copy)
            else:
                nc.vector.tensor_copy(att_slice, po[0:64, :])
                nc.vector.tensor_copy(sums[:], po[64:65, :])
            recip = norm_pool.tile([1, 512], f32, tag="recip", name="recip")
            nc.vector.reciprocal_approx_fast(out=recip[:], in_=sums[:])
            bcast = norm_pool.tile([128, 512], f32, tag="bcast", name="bcast")
            nc.gpsimd.partition_broadcast(bcast[:], recip[:])
            nc.vector.tensor_mul(att_slice, att_slice, bcast[rr:rr + 64, :])

        def attn_duo(qc, di):
            """Generator: yields at PE-filler points."""
            hA, hB = 2 * di, 2 * di + 1
            nblk = 4 * qc + 4
            poA = ps_o.tile([65, 512], f32, tag="pso", name="poA")
            poB = ps_o.tile([65, 512], f32, tag="pso", name="poB")
            ets = {}

            def emit_pv(j):
                et = ets.pop(j)
                lo = max(0, (j - 4 * qc)) * 128
                for idx, (h, po) in enumerate(((hA, poA), (hB, poB))):
                    nc.tensor.matmul(
                        po[:, lo:512],
                        lhsT=v_view[:, j, h, :],
                        rhs=et[:, idx, lo:512],
                        start=(j == 0), stop=(j == nblk - 1),
                    )

            for j in range(nblk):
                pss = ps_s.tile([128, 2, 512], f32, tag="pss", name="pss")
                for idx, rr in enumerate((0, 64)):
                    nc.tensor.matmul(
                        pss[:, idx, :],
                        lhsT=sb_kT[rr:rr + 64, di, j * 128:(j + 1) * 128],
                        rhs=sb_qT[rr:rr + 64, di, qc * 512:(qc + 1) * 512],
                        start=True, stop=True,
                        tile_position=(rr, 0),
                    )
                et = exp_pool.tile([128, 2, 512], bf16, tag="expT", name="et")
                lo = max(0, (j - 4 * qc)) * 128
                nc.scalar.activation(et[:, :, lo:512], pss[:, :, lo:512],
                                     Exp, scale=0.125)
                if j >= 4 * qc:
                    nc.vector.tensor_mul(et[:, :, lo:lo + 128],
                                         et[:, :, lo:lo + 128], sb_mask2[:])
                ets[j] = et
                if j >= 1:
                    yield
                    emit_pv(j - 1)
                else:
                    yield
            emit_pv(nblk - 1)
            last = (qc == 3 and di == 3)
            norm_store(poA, 0, di, qc)
            norm_store(poB, 64, di, qc, on_act=last)
            yield
            if qc > 0:
                emit_proj((qc - 1) * 4 + di)
                yield

        # ---- fused schedule ----
        # wpT / mask admission chained behind round-2 projection traffic
        def admit_late():
            prev = None
            for m in range(2):
                dma = nc.sync.dma_start(sb_mask2[:, m, :], mk_d[:])
                add_dep_helper(dma.ins,
                               (round_copy[0] if prev is None else prev).ins,
                               sync=False, reason="admit trimask early")
                prev = dma
            for ko in range(4):
                dma = nc.sync.dma_start(sb_wpT[:, ko, :], wpT_r[:, ko, :])
                add_dep_helper(dma.ins, prev.ins, sync=False,
                               reason="admit wpT after mask")
                prev = dma

        stage_x(1)
        # tci0 minimal prefix: exactly what attention (qc0, duo0) needs —
        # kT pair 0 (fb4), qT pair 0 (fb0), all four v blocks. The remaining
        # tci0 groups go to the filler queue so duo d's needs (fb 4+d, fb d)
        # cascade in as earlier duos run.
        for fb in (4, 0):
            qk_group(0, fb)
        for tb in range(4):
            v_group(0, tb)
        admit_late()
        stage_x(2)

        fillers = deque()
        for fb in (5, 1, 6, 2, 7, 3):
            fillers.append((0, lambda f=fb: qk_group(0, f)))
        for tci in (1, 2, 3):
            fillers.extend(groups_for(tci))
        # x chunk 3 staged when tci=2 groups begin
        staged3 = [False]

        def pop_filler():
            tci, g = fillers.popleft()
            if tci == 2 and not staged3[0]:
                staged3[0] = True
                stage_x(3)
            g()

        tick = [0]

        def filler_tick():
            tick[0] += 1
            if tick[0] % 3 == 0 and fillers:
                pop_filler()

        for qc in range(4):
            while fillers and fillers[0][0] <= qc:
                pop_filler()
            for di in range(4):
                for _ in attn_duo(qc, di):
                    filler_tick()
            if qc == 3:
                for tblk in range(12, 16):
                    emit_proj(tblk, on_act=True)

    nc.compile()
    return nc


def _get_module():
    if "nc" not in _CACHE:
        _CACHE["nc"] = _build_module()
    return _CACHE["nc"]


def _make_trimask():
    # trimask[kk, q] = 1 iff q >= kk (diagonal 128x128 block)
    q = np.arange(128)[None, :]
    kk = np.arange(128)[:, None]
    return (q >= kk).astype(np.float32)


def make_in_maps(x, W_qkv, W_proj):
    import ml_dtypes

    bf16 = ml_dtypes.bfloat16
    x = np.asarray(x, dtype=np.float32)
    W_qkv = np.asarray(W_qkv, dtype=np.float32)
    W_proj = np.asarray(W_proj, dtype=np.float32)
    trimask = _make_trimask().astype(bf16)
    in_maps = []
    for c in range(N_CORES):
        b, g = c // 2, c % 2
        s = 512 * g
        wqk = np.concatenate([W_qkv[s:s + 512], W_qkv[1024 + s:1024 + s + 512]], 0)
        in_maps.append({
            "xT": np.ascontiguousarray(x[b].T).astype(bf16),
            "wqkT": np.ascontiguousarray(wqk.T).astype(bf16),
            "wvT": np.ascontiguousarray(W_qkv[2048 + s:2048 + s + 512].T).astype(bf16),
            "wpT": np.ascontiguousarray(W_proj[:, s:s + 512].T).astype(bf16),
            "trimask": trimask,
        })
    return in_maps


def run(x, W_qkv, W_proj, trace=False):
    """Returns (y_full [4,2048,1024], BassKernelResults)."""
    from concourse import bass_utils

    nc = _get_module()
    in_maps = make_in_maps(x, W_qkv, W_proj)
    res = bass_utils.run_bass_kernel_spmd(
        nc, in_maps, core_ids=list(range(N_CORES)), trace=trace)
    y = np.zeros((4, T, 1024), np.float32)
    for b in range(4):
        y[b] = res.results[2 * b]["y"] + res.results[2 * b + 1]["y"]
    return y, res


def kernel(x, W_qkv, W_proj):
    y, _ = run(x, W_qkv, W_proj, trace=False)
    return y


# revision 10
# speedup vs baseline: 1.0197x; 1.0197x over previous
"""Causal self-attention (B=4, T=2048, C=1024, H=16, D=64) on 8 TRN2 NeuronCores.

Sharding: 4 batches x 2 head-groups (8 heads each). Core c handles batch c//2,
heads 8*(c%2) .. 8*(c%2)+7. Host pre-transposes x and slices/transposes the
weights so the device kernel needs no on-chip transposes. All matmul operands
are bf16 (fp32 PSUM accumulation).

Fused single-pass schedule: the QKV projection (4 t-chunks x 12 matmul groups)
and the attention/proj work are interleaved in one stream. Attention for query
chunk qc starts as soon as projection chunk tci=qc is done; remaining
projection groups are rationed into the attention loop as PE filler so the
scalar engine (exp) and PE stay busy together instead of running as separate
phases.

  projection: qkT[feat, T] = Wqk_g @ x^T, V[t, vfeat] = x @ Wv_g^T (+ ones col)
  attention:  per head-pair (2p, 2p+1) and query chunk qc: for each k-block j,
              S^T blocks for BOTH heads land in one [128,2,512] PSUM tile via
              row-tiled matmuls (head 2p on PE tile (0,0) from SBUF partitions
              0:64, head 2p+1 on (64,0) — concurrent); ONE exp call covers the
              pair, so both heads' next scores are gated by the same semaphore
              and issue back-to-back (keeps the pairing aligned).
              P^T = exp(S^T/8) * causal masks, out^T[d|sum, q] = [V_h|1]^T P^T,
              normalized by approx-reciprocal + gpsimd partition-broadcast.
  proj:       y = attn^T.T @ Wp_g^T, interleaved between pairs as PE filler

Each core returns a [2048, 1024] partial; the host sums the two head-group
partials per batch.
"""

import numpy as np

T = 2048
N_CORES = 8

_CACHE = {}


def _build_module():
    from collections import deque
    from contextlib import ExitStack

    import concourse.tile as tile
    from concourse.tile_rust import add_dep_helper
    from concourse import bacc, mybir

    f32 = mybir.dt.float32
    bf16 = mybir.dt.bfloat16
    Exp = mybir.ActivationFunctionType.Exp
    Copy = mybir.ActivationFunctionType.Copy

    nc = bacc.Bacc("TRN2", target_bir_lowering=False, debug=False,
                   num_devices=N_CORES)

    xT_d = nc.dram_tensor("xT", (1024, 2048), bf16, kind="ExternalInput").ap()
    wqkT_d = nc.dram_tensor("wqkT", (1024, 1024), bf16, kind="ExternalInput").ap()
    wvT_d = nc.dram_tensor("wvT", (1024, 512), bf16, kind="ExternalInput").ap()
    wpT_d = nc.dram_tensor("wpT", (512, 1024), bf16, kind="ExternalInput").ap()
    mk_d = nc.dram_tensor("trimask", (128, 128), bf16, kind="ExternalInput").ap()
    y_d = nc.dram_tensor("y", (2048, 1024), f32, kind="ExternalOutput").ap()

    with tile.TileContext(nc) as tc, ExitStack() as ctx:
        pers = ctx.enter_context(tc.tile_pool(name="pers", bufs=1))
        sb_qT = pers.tile([128, 4, 2048], bf16, name="sb_qT")
        sb_kT = pers.tile([128, 4, 2048], bf16, name="sb_kT")
        sb_v = pers.tile([128, 16, 520], bf16, name="sb_v")
        v_view = sb_v[:].rearrange("p t (h e) -> p t h e", e=65)
        sb_attnT = pers.tile([128, 4, 2048], bf16, name="sb_attnT")
        sb_wpT = pers.tile([128, 4, 1024], bf16, name="sb_wpT")
        sb_mask2 = pers.tile([128, 2, 128], bf16, name="sb_mask2")
        sb_wvT = pers.tile([128, 8, 512], bf16, name="sb_wvT")

        wqk_pool = ctx.enter_context(tc.tile_pool(name="wqk", bufs=8))
        xt_pool = ctx.enter_context(tc.tile_pool(name="xt", bufs=2))
        ps_misc = ctx.enter_context(tc.tile_pool(name="ps_misc", bufs=2,
                                                 space="PSUM"))
        ps_s = ctx.enter_context(tc.tile_pool(name="ps_s", bufs=2, space="PSUM"))
        ps_o = ctx.enter_context(tc.tile_pool(name="ps_o", bufs=2, space="PSUM"))
        exp_pool = ctx.enter_context(tc.tile_pool(name="expp", bufs=3))
        norm_pool = ctx.enter_context(tc.tile_pool(name="normp", bufs=3))
        y_pool = ctx.enter_context(tc.tile_pool(name="yp", bufs=2))
        misc_sb = ctx.enter_context(tc.tile_pool(name="miscsb", bufs=1))

        wqkT_r = wqkT_d.rearrange("(co ci) f -> ci co f", ci=128)
        xT_r = xT_d.rearrange("(co ci) t -> ci co t", ci=128)
        wvT_r = wvT_d.rearrange("(co ci) f -> ci co f", ci=128)
        wpT_r = wpT_d.rearrange("(ko ki) n -> ki ko n", ki=128)

        # ---- DMA admission, ordered to match consumption:
        # fb4 (kT pair0) -> x0 -> fb0 (qT pair0) -> causal masks (first PVs)
        # -> fb5, fb1 (duo1) -> wvT (v groups) -> fb6, fb2 -> fb7, fb3 ----
        fbs = [4, 0, 5, 1, 6, 2, 7, 3]
        wtiles = {}
        wt = wqk_pool.tile([128, 8, 128], bf16, tag="wqk", name="wt4")
        nc.sync.dma_start(wt[:], wqkT_r[:, :, 4 * 128:5 * 128])
        wtiles[4] = wt
        xchunks = {}
        xchunks[0] = xt_pool.tile([128, 8, 512], bf16, tag="xt", name="xc0")
        xdmas = [nc.sync.dma_start(xchunks[0][:, co, :], xT_r[:, co, 0:512])
                 for co in range(8)]
        prev = xdmas[-1]

        def _stage_w(fb, prev):
            wt = wqk_pool.tile([128, 8, 128], bf16, tag="wqk", name=f"wt{fb}")
            d = nc.sync.dma_start(wt[:], wqkT_r[:, :, fb * 128:(fb + 1) * 128])
            add_dep_helper(d.ins, prev.ins, sync=False, reason="stage w")
            wtiles[fb] = wt
            return d

        prev = _stage_w(0, prev)
        for m in range(2):
            d = nc.sync.dma_start(sb_mask2[:, m, :], mk_d[:])
            add_dep_helper(d.ins, prev.ins, sync=False, reason="stage mask")
            prev = d
        for fb in (5, 1):
            prev = _stage_w(fb, prev)
        for co in range(8):
            d = nc.sync.dma_start(sb_wvT[:, co, :], wvT_r[:, co, :])
            add_dep_helper(d.ins, prev.ins, sync=False, reason="stage wv")
            prev = d
        for fb in (6, 2, 7, 3):
            prev = _stage_w(fb, prev)
        wv_last = prev

        # ones column of sb_v via exp(0)=1 — also warms the ACT exp table
        zeros = misc_sb.tile([128, 128], f32, name="zeros")
        nc.vector.memset(zeros[:], 0.0)
        nc.scalar.activation(
            v_view[:, :, :, 64:65],
            zeros[:].rearrange("p (a b c) -> p a b c", a=16, b=8),
            Exp,
        )

        # ---- projection group emitters ----
        round_copy = {}
        x_stage_prev = [wv_last]

        def stage_x(tci):
            # admit x chunk `tci` (chained behind earlier admission DMAs)
            xchunks[tci] = xt_pool.tile([128, 8, 512], bf16, tag="xt",
                                        name=f"xc{tci}")
            first = None
            for co in range(8):
                d = nc.sync.dma_start(
                    xchunks[tci][:, co, :],
                    xT_r[:, co, tci * 512:(tci + 1) * 512])
                if first is None:
                    add_dep_helper(d.ins, x_stage_prev[0].ins, sync=False,
                                   reason="stage x chunk")
                    first = d
            x_stage_prev[0] = d

        def qk_group(tci, fb):
            dst, pblk = (sb_kT, fb - 4) if fb >= 4 else (sb_qT, fb)
            ps = ps_misc.tile([128, 512], f32, tag="psb", name="psqk")
            for co in range(8):
                nc.tensor.matmul(
                    ps[:],
                    lhsT=wtiles[fb][:, co, :],
                    rhs=xchunks[tci][:, co, :],
                    start=(co == 0), stop=(co == 7),
                )
            cp = nc.vector.tensor_copy(
                dst[:, pblk, tci * 512:(tci + 1) * 512], ps[:])
            if fb == 4:
                round_copy[tci] = cp

        def v_group(tci, tb):
            tblk = tci * 4 + tb
            ps = ps_misc.tile([128, 512], f32, tag="psb", name="psv")
            for co in range(8):
                nc.tensor.matmul(
                    ps[:],
                    lhsT=xchunks[tci][:, co, tb * 128:(tb + 1) * 128],
                    rhs=sb_wvT[:, co, :],
                    start=(co == 0), stop=(co == 7),
                )
            nc.vector.tensor_copy(
                v_view[:, tblk, :, 0:64],
                ps[:].rearrange("p (h d) -> p h d", d=64),
            )

        def groups_for(tci):
            gs = []
            for fb in fbs:
                gs.append((tci, lambda t=tci, f=fb: qk_group(t, f)))
            for tb in range(4):
                gs.append((tci, lambda t=tci, b=tb: v_group(t, b)))
            return gs

        # ---- attention emitters ----
        def emit_proj(tblk, on_act=False):
            for n in range(2):
                ysb = y_pool.tile([128, 512], f32, tag="ysb", name="ysb")
                pj = ps_misc.tile([128, 512], f32, tag="psb", name="pj")
                for ko in range(4):
                    nc.tensor.matmul(
                        pj[:],
                        lhsT=sb_attnT[:, ko, tblk * 128:(tblk + 1) * 128],
                        rhs=sb_wpT[:, ko, n * 512:(n + 1) * 512],
                        start=(ko == 0), stop=(ko == 3),
                    )
                if on_act:
                    nc.scalar.activation(ysb[:], pj[:], Copy)
                else:
                    nc.vector.tensor_copy(ysb[:], pj[:])
                nc.sync.dma_start(
                    y_d[tblk * 128:(tblk + 1) * 128, n * 512:(n + 1) * 512],
                    ysb[:])

        def norm_store(po, rr, p_, qc, on_act=False):
            att_slice = sb_attnT[rr:rr + 64, p_, qc * 512:(qc + 1) * 512]
            sums = norm_pool.tile([1, 512], f32, tag="sums", name="sums")
            if on_act:
                nc.scalar.activation(att_slice, po[0:64, :], Copy)
                nc.scalar.activation(sums[:], po[64:65, :], Copy)
            else:
                nc.vector.tensor_copy(att_slice, po[0:64, :])
                nc.vector.tensor_copy(sums[:], po[64:65, :])
            recip = norm_pool.tile([1, 512], f32, tag="recip", name="recip")
            nc.vector.reciprocal_approx_fast(out=recip[:], in_=sums[:])
            bcast = norm_pool.tile([128, 512], f32, tag="bcast", name="bcast")
            nc.gpsimd.partition_broadcast(bcast[:], recip[:])
            nc.vector.tensor_mul(att_slice, att_slice, bcast[rr:rr + 64, :])

        def attn_duo(qc, di):
            """Generator: yields at PE-filler points."""
            hA, hB = 2 * di, 2 * di + 1
            nblk = 4 * qc + 4
            poA = ps_o.tile([65, 512], f32, tag="pso", name="poA")
            poB = ps_o.tile([65, 512], f32, tag="pso", name="poB")
            ets = {}

            def emit_pv(j):
                et = ets.pop(j)
                lo = max(0, (j - 4 * qc)) * 128
                for idx, (h, po) in enumerate(((hA, poA), (hB, poB))):
                    nc.tensor.matmul(
                        po[:, lo:512],
                        lhsT=v_view[:, j, h, :],
                        rhs=et[:, idx, lo:512],
                        start=(j == 0), stop=(j == nblk - 1),
                    )

            for j in range(nblk):
                pss = ps_s.tile([128, 2, 512], f32, tag="pss", name="pss")
                for idx, rr in enumerate((0, 64)):
                    nc.tensor.matmul(
                        pss[:, idx, :],
                        lhsT=sb_kT[rr:rr + 64, di, j * 128:(j + 1) * 128],
                        rhs=sb_qT[rr:rr + 64, di, qc * 512:(qc + 1) * 512],
                        start=True, stop=True,
                        tile_position=(rr, 0),
                    )
                et = exp_pool.tile([128, 2, 512], bf16, tag="expT", name="et")
                lo = max(0, (j - 4 * qc)) * 128
                nc.scalar.activation(et[:, :, lo:512], pss[:, :, lo:512],
                                     Exp, scale=0.125)
                if j >= 4 * qc:
                    nc.vector.tensor_mul(et[:, :, lo:lo + 128],
                                         et[:, :, lo:lo + 128], sb_mask2[:])
                ets[j] = et
                if j >= 1:
                    yield
                    emit_pv(j - 1)
                else:
                    yield
            emit_pv(nblk - 1)
            last = (qc == 3 and di == 3)
            norm_store(poA, 0, di, qc)
            norm_store(poB, 64, di, qc, on_act=last)
            yield
            if qc > 0:
                emit_proj((qc - 1) * 4 + di)
                yield

        # ---- fused schedule ----
        # wpT / mask admission chained behind round-2 projection traffic
        def admit_late():
            prev = wv_last
            for ko in range(4):
                dma = nc.sync.dma_start(sb_wpT[:, ko, :], wpT_r[:, ko, :])
                add_dep_helper(dma.ins, prev.ins, sync=False,
                               reason="admit wpT after weights")
                prev = dma

        # tci0 minimal prefix: exactly what attention (qc0, duo0) needs —
        # kT pair 0 (fb4), qT pair 0 (fb0). The remaining tci0 groups go to
        # the filler queue so duo d's needs (fb 4+d, fb d) and the v blocks
        # cascade in as earlier duos run (matching DMA arrival order).
        for fb in (4, 0):
            qk_group(0, fb)
        admit_late()
        stage_x(1)
        stage_x(2)

        fillers = deque()
        fillers.append((0, lambda: qk_group(0, 5)))
        fillers.append((0, lambda: qk_group(0, 1)))
        for tb in range(4):
            fillers.append((0, lambda b=tb: v_group(0, b)))
        for fb in (6, 2, 7, 3):
            fillers.append((0, lambda f=fb: qk_group(0, f)))
        for tci in (1, 2, 3):
            fillers.extend(groups_for(tci))
        # x chunk 3 staged when tci=2 groups begin
        staged3 = [False]

        def pop_filler():
            tci, g = fillers.popleft()
            if tci == 2 and not staged3[0]:
                staged3[0] = True
                stage_x(3)
            g()

        tick = [0]

        def filler_tick():
            tick[0] += 1
            if tick[0] % 3 == 0 and fillers:
                pop_filler()

        for qc in range(4):
            while fillers and fillers[0][0] <= qc:
                pop_filler()
            for di in range(4):
                for _ in attn_duo(qc, di):
                    filler_tick()
            if qc == 3:
                for tblk in range(12, 16):
                    emit_proj(tblk, on_act=True)

    nc.compile()
    return nc


def _get_module():
    if "nc" not in _CACHE:
        _CACHE["nc"] = _build_module()
    return _CACHE["nc"]


def _make_trimask():
    # trimask[kk, q] = 1 iff q >= kk (diagonal 128x128 block)
    q = np.arange(128)[None, :]
    kk = np.arange(128)[:, None]
    return (q >= kk).astype(np.float32)


def make_in_maps(x, W_qkv, W_proj):
    import ml_dtypes

    bf16 = ml_dtypes.bfloat16
    x = np.asarray(x, dtype=np.float32)
    W_qkv = np.asarray(W_qkv, dtype=np.float32)
    W_proj = np.asarray(W_proj, dtype=np.float32)
    trimask = _make_trimask().astype(bf16)
    in_maps = []
    for c in range(N_CORES):
        b, g = c // 2, c % 2
        s = 512 * g
        wqk = np.concatenate([W_qkv[s:s + 512], W_qkv[1024 + s:1024 + s + 512]], 0)
        in_maps.append({
            "xT": np.ascontiguousarray(x[b].T).astype(bf16),
            "wqkT": np.ascontiguousarray(wqk.T).astype(bf16),
            "wvT": np.ascontiguousarray(W_qkv[2048 + s:2048 + s + 512].T).astype(bf16),
            "wpT": np.ascontiguousarray(W_proj[:, s:s + 512].T).astype(bf16),
            "trimask": trimask,
        })
    return in_maps


def run(x, W_qkv, W_proj, trace=False):
    """Returns (y_full [4,2048,1024], BassKernelResults)."""
    from concourse import bass_utils

    nc = _get_module()
    in_maps = make_in_maps(x, W_qkv, W_proj)
    res = bass_utils.run_bass_kernel_spmd(
        nc, in_maps, core_ids=list(range(N_CORES)), trace=trace)
    y = np.zeros((4, T, 1024), np.float32)
    for b in range(4):
        y[b] = res.results[2 * b]["y"] + res.results[2 * b + 1]["y"]
    return y, res


def kernel(x, W_qkv, W_proj):
    y, _ = run(x, W_qkv, W_proj, trace=False)
    return y
